# revision 1
# baseline (speedup 1.0000x reference)
"""NanoLLM (Mamba-ish, d_state=1, d_conv=1) Trainium2 kernel, 8 NeuronCores.

Sharding: core c owns batch c//2, L-half c%2 (1024 tokens). Weights are
replicated; every matmul is token-parallel. The L-recurrence runs per-core
via tensor_tensor_scan (h = a*h + b along the free dim); the cross-core
state handoff (even -> odd core of each batch pair) is a tiny pairwise
AllGather of the final scan state plus a cumprod fixup folded into y:
    y*silu(z) = yb3 + s * cf2
      yb3 = (h_loc*Cm + D*xi)*silu(z),  cf2 = (cumprod(a)*Cm)*silu(z)
with s = (peer final state) * parity. SPMD-symmetric, no control flow.

Precision: bf16 hi/lo 3-pass split matmuls (~fp32-equivalent) for
emb/in/out projections; plain bf16 for x_proj/dt_proj/head; fp32
elementwise + scan. Measured vs fp64 numpy: ~0.23% max rel err — equal to
the intrinsic fp32 envelope of this (activation-collapsing) model.

Layout: feature-major activations [128 feat, 512 tok] tiles, two token
chunks per core. Weights are matmul lhsT in natural [K, M] layout;
emb/in/out projections host-pre-tiled into [K/128, M/128, 128, 128]
blocks for contiguous DMA. x is spilled to DRAM between layers (SBUF
pressure). The head flips to token-major PSUM for contiguous output DMA.
"""

import numpy as np
from ml_dtypes import bfloat16 as np_bf16

import concourse.bass as bass
import concourse.bacc as bacc
import concourse.mybir as mybir
import concourse.tile as tile
from concourse.bass_utils import run_bass_kernel_spmd

NCORES = 8
B, L, V, EMB, D = 4, 2048, 6400, 512, 768
ED, DT_RANK = 768, 48
XCOLS, BCOL, CCOL, DTCOL = 112, 0, 32, 64  # padded x_proj layout
NLAYERS = 8
T = 1024          # tokens per core
P = 128
NF = 512          # token chunk
NCH = T // NF     # 2
KD = D // P       # 6
KE = ED // P      # 6
KM = EMB // P     # 4
EPS = 1e-6

f32 = mybir.dt.float32
bf16 = mybir.dt.bfloat16
AX = mybir.AluOpType
AF = mybir.ActivationFunctionType

RG_PAIRS = [[0, 1], [2, 3], [4, 5], [6, 7]]


def build_program(n_layers=NLAYERS, sim_safe=False, no_cc=False):
    nc = bacc.Bacc(
        "TRN2",
        target_bir_lowering=False,
        debug=False,
        enable_asserts=False,
        num_devices=NCORES,
    )

    def inp(name, shape, dt):
        return nc.dram_tensor(name, shape, dt, kind="ExternalInput").ap()

    g = dict(
        e_hi=inp("e_hi", [EMB, T], bf16),
        e_lo=inp("e_lo", [EMB, T], bf16),
        w_emb_hi=inp("w_emb_hi", [KM, KD, P, P], bf16),
        w_emb_lo=inp("w_emb_lo", [KM, KD, P, P], bf16),
        w_in_hi=inp("w_in_hi", [KD, 2 * KE, P, P], bf16),
        w_in_lo=inp("w_in_lo", [KD, 2 * KE, P, P], bf16),
        w_out_hi=inp("w_out_hi", [KE, KD, P, P], bf16),
        w_out_lo=inp("w_out_lo", [KE, KD, P, P], bf16),
        w_x=inp("w_x", [ED, XCOLS], bf16),
        w_dt=inp("w_dt", [DT_RANK, ED], bf16),
        w_head=inp("w_head", [D, V], bf16),
        # packed per-channel params [768, 8]: 0 norm_w, 1 conv_w, 2 conv_b,
        # 3 A0=-exp(A_log), 4 D_param, 5 dt_b, 6 out_norm_w, 7 pad
        pp=inp("pp", [D, 8], f32),
        head_b=inp("head_b", [1, V], f32),
        parity=inp("parity", [P, 1], f32),
        out=nc.dram_tensor("out", [T, V], f32, kind="ExternalOutput").ap(),
        x_dram=[
            nc.dram_tensor(f"x_dram{i}", [KD, P, T], f32, kind="Internal").ap()
            for i in range(2)
        ],
        cc_in=[
            nc.dram_tensor(f"cc_in{k}", [P, KE], f32, kind="Internal").ap()
            for k in range(n_layers)
        ],
        cc_out=[
            nc.dram_tensor(f"cc_out{k}", [2, P, KE], f32,
                           kind="Internal").ap()
            for k in range(n_layers)
        ],
    )

    with tile.TileContext(nc) as tc:
        _body(nc, tc, g, n_layers, sim_safe, no_cc)

    nc.compile()
    return nc


def _body(nc, tc, g, n_layers, sim_safe=False, no_cc=False):
    from contextlib import ExitStack

    with ExitStack() as ctx:
        cpool = ctx.enter_context(tc.tile_pool(name="consts", bufs=1))
        ap_ = ctx.enter_context(tc.tile_pool(name="act", bufs=1))
        psp = ctx.enter_context(
            tc.tile_pool(name="ps", bufs=1, space=bass.MemorySpace.PSUM))

        dma = nc.sync.dma_start
        uid = [0]

        def fresh(tag, shape=(P, NF), dt=f32, bufs=None, pool=None):
            uid[0] += 1
            dflt = {"gf": 3, "gb": 2}.get(tag.split("_")[0], 1)
            return (pool or ap_).tile(
                list(shape), dt, name=f"t{uid[0]}_{tag}", tag=tag,
                bufs=bufs if bufs is not None else dflt)

        # ---------------- constants ----------------
        ones1 = fresh("ones1", (P, 1), bf16, pool=cpool)
        nc.vector.memset(ones1, 1.0)
        zeros = fresh("zeros", (P, NF), f32, pool=cpool)
        nc.vector.memset(zeros, 0.0)
        par_sb = fresh("par_sb", (P, 1), f32, pool=cpool)
        dma(par_sb, g["parity"])
        epsc = fresh("epsc", (1, 1), f32, pool=cpool)
        nc.vector.memset(epsc, EPS)
        pp_sb = []
        for d in range(KD):
            t = fresh(f"pp_{d}", (P, 8), f32, pool=cpool)
            dma(t, g["pp"][d * P:(d + 1) * P, :])
            pp_sb.append(t)
        NW = [t[:, 0:1] for t in pp_sb]
        CW = [t[:, 1:2] for t in pp_sb]
        CB = [t[:, 2:3] for t in pp_sb]
        A0 = [t[:, 3:4] for t in pp_sb]
        DPr = [t[:, 4:5] for t in pp_sb]
        DTB = [t[:, 5:6] for t in pp_sb]
        ONW = [t[:, 6:7] for t in pp_sb]

        wx_sb = []
        for k in range(KE):
            t = fresh(f"wx_{k}", (P, XCOLS), bf16, pool=cpool)
            dma(t, g["w_x"][k * P:(k + 1) * P, :])
            wx_sb.append(t)
        wdt_sb = fresh("wdt", (DT_RANK, ED), bf16, pool=cpool)
        dma(wdt_sb, g["w_dt"])

        # ---------------- helpers ----------------
        def load_x(src, c):
            # x tiles live in the per-d gf rings: released after hf (the
            # stt in rmsnorm_cast), so {xs, sq, hf} fits the 3-slot ring.
            xs = []
            for d in range(KD):
                t = fresh(f"gf_{d}")
                dma(t, src[d, :, c * NF:(c + 1) * NF])
                xs.append(t)
            return xs

        def rmsnorm_cast(x_tiles, wvec, want_lo, hh_tag):
            ps = fresh("norm_ps", (1, NF), f32, bufs=2, pool=psp)
            n_mm = 0
            for d in range(KD):
                sq = fresh(f"gf_{d}")
                nc.scalar.square(sq, x_tiles[d])
                sqh = fresh(f"gb_{d}", dt=bf16)
                nc.vector.tensor_copy(sqh, sq)
                sql = fresh(f"gb_{d}", dt=bf16)
                nc.vector.tensor_sub(sql, sq, sqh)
                for opnd in (sqh, sql):
                    nc.tensor.matmul(ps, ones1, opnd,
                                     start=(n_mm == 0), stop=(n_mm == 2 * KD - 1))
                    n_mm += 1
            rstd = fresh("rstd", (1, NF))
            nc.scalar.activation(rstd, ps, AF.Sqrt, bias=epsc, scale=1.0 / D)
            rinv = fresh("rinv", (1, NF))
            nc.vector.reciprocal(rinv, rstd)
            scl = fresh("bc_scl", bufs=1)
            nc.gpsimd.partition_broadcast(scl, rinv)
            hh, hl = [], []
            for d in range(KD):
                hf = fresh(f"gf_{d}")
                nc.vector.scalar_tensor_tensor(
                    hf, x_tiles[d], wvec[d], scl, AX.mult, AX.mult)
                h1 = fresh(f"{hh_tag}_{d}", dt=bf16, bufs=1)
                nc.vector.tensor_copy(h1, hf)
                hh.append(h1)
                if want_lo:
                    h2 = fresh(f"hl_{d}", dt=bf16, bufs=1)
                    nc.vector.tensor_sub(h2, hf, h1)
                    hl.append(h2)
            return hh, hl

        def load_blocks(wblk_hi, wblk_lo, m, nk):
            blks = []
            for k in range(nk):
                bh = fresh("wbi", (P, P), bf16, bufs=24)
                dma(bh, wblk_hi[k, m])
                bl = fresh("wbi", (P, P), bf16, bufs=24)
                dma(bl, wblk_lo[k, m])
                blks.append((bh, bl))
            return blks

        def mm3(ps, blks, rhi, rlo):
            nk = len(blks)
            n = 0
            for k in range(nk):
                bh, bl = blks[k]
                for wt, rt in ((bh, rhi[k]), (bl, rhi[k]), (bh, rlo[k])):
                    nc.tensor.matmul(ps, wt, rt,
                                     start=(n == 0), stop=(n == 3 * nk - 1))
                    n += 1

        # ---------------- embedding projection -> x_dram[0] ----------------
        for c in range(NCH):
            csl = slice(c * NF, (c + 1) * NF)
            ehi_sb, elo_sb = [], []
            for k in range(KM):
                th = fresh("eb", dt=bf16, bufs=8)
                dma(th, g["e_hi"][k * P:(k + 1) * P, csl])
                tm = fresh("eb", dt=bf16, bufs=8)
                dma(tm, g["e_lo"][k * P:(k + 1) * P, csl])
                ehi_sb.append(th)
                elo_sb.append(tm)
            for m in range(KD):
                blks = load_blocks(g["w_emb_hi"], g["w_emb_lo"], m, KM)
                ps = fresh("mm_ps", pool=psp, bufs=4)
                mm3(ps, blks, ehi_sb, elo_sb)
                xt = fresh("xio", bufs=3)
                nc.vector.tensor_copy(xt, ps)
                dma(g["x_dram"][0][m, :, c * NF:(c + 1) * NF], xt)

        # ---------------- layers ----------------
        for ly in range(n_layers):
            src = g["x_dram"][ly % 2]
            dst = g["x_dram"][(ly + 1) % 2]

            yb3 = [[None] * KE for _ in range(NCH)]
            cf2 = [[None] * KE for _ in range(NCH)]
            siluz_keep = [None] * NCH  # noqa: F841 (lifetime via yb3/cf2)
            hl_last = [None] * KE
            cp_last = [None] * KE

            for c in range(NCH):
                xs = load_x(src, c)
                hh, hl = rmsnorm_cast(xs, NW, True, "hh")

                xi = [None] * KE
                xih = [None] * KE
                sz = [None] * KE
                for m in range(2 * KE):
                    blks = load_blocks(g["w_in_hi"], g["w_in_lo"], m, KD)
                    ps = fresh("mm_ps", pool=psp, bufs=4)
                    mm3(ps, blks, hh, hl)
                    if m < KE:
                        t = fresh(f"xi_{m}", bufs=1)
                        if sim_safe:
                            sg = fresh(f"gf_{m}")
                            nc.scalar.activation(sg, ps, AF.Sigmoid,
                                                 bias=CB[m], scale=CW[m])
                            v = fresh(f"gf_{m}")
                            nc.vector.tensor_scalar(v, ps, CW[m], CB[m],
                                                    AX.mult, AX.add)
                            nc.vector.tensor_mul(t, v, sg)
                        else:
                            nc.scalar.activation(t, ps, AF.Silu,
                                                 bias=CB[m], scale=CW[m])
                        xi[m] = t
                        t = fresh(f"gb_{m}", dt=bf16)
                        if sim_safe:
                            nc.vector.tensor_copy(t, xi[m])
                        else:
                            nc.scalar.activation(t, ps, AF.Silu,
                                                 bias=CB[m], scale=CW[m])
                        xih[m] = t
                    else:
                        t = fresh(f"sz_{m - KE}", bufs=1)
                        if sim_safe:
                            sg = fresh(f"gf_{m - KE}")
                            nc.scalar.activation(sg, ps, AF.Sigmoid)
                            nc.vector.tensor_mul(t, ps, sg)
                        else:
                            nc.scalar.activation(t, ps, AF.Silu)
                        sz[m - KE] = t

                dps = fresh("d_ps", (XCOLS, NF), pool=psp, bufs=1)
                for k in range(KE):
                    nc.tensor.matmul(dps, wx_sb[k], xih[k],
                                     start=(k == 0), stop=(k == KE - 1))
                dtr = fresh("dtr", (DT_RANK, NF), bf16, bufs=1)
                nc.vector.tensor_copy(dtr, dps[DTCOL:DTCOL + DT_RANK, :])
                bmr = fresh("bmr", (1, NF))
                nc.vector.tensor_copy(bmr, dps[BCOL:BCOL + 1, :])
                cmr = fresh("cmr", (1, NF))
                nc.vector.tensor_copy(cmr, dps[CCOL:CCOL + 1, :])
                bm_b = fresh("bc_bm", bufs=1)
                nc.gpsimd.partition_broadcast(bm_b, bmr)
                cm_b = fresh("bc_cm", bufs=1)
                nc.gpsimd.partition_broadcast(cm_b, cmr)

                for m in range(KE):
                    ps = fresh("mm_ps", pool=psp, bufs=4)
                    nc.tensor.matmul(ps, wdt_sb[:, m * P:(m + 1) * P], dtr,
                                     start=True, stop=True)
                    # softplus(u) = ln(exp(u) + 1)  (no softplus LUT on TRN2)
                    eu = fresh(f"gf_{m}")
                    nc.scalar.activation(eu, ps, AF.Exp, bias=DTB[m])
                    delta = fresh(f"gf_{m}")
                    nc.scalar.activation(delta, eu, AF.Ln, bias=1.0)
                    at = fresh(f"gf_{m}")
                    nc.scalar.activation(at, delta, AF.Exp, scale=A0[m])
                    bt = fresh(f"gf_{m}")
                    nc.vector.tensor_mul(bt, delta, xi[m])
                    bt2 = fresh(f"gf_{m}")
                    nc.vector.tensor_mul(bt2, bt, bm_b)

                    hs = fresh(f"gf_{m}")
                    nc.vector.tensor_tensor_scan(
                        hs, at, bt2,
                        initial=(0.0 if c == 0 else hl_last[m]),
                        op0=AX.mult, op1=AX.add)
                    cp = fresh(f"gf_{m}")
                    nc.vector.tensor_tensor_scan(
                        cp, at, zeros,
                        initial=(1.0 if c == 0 else cp_last[m]),
                        op0=AX.mult, op1=AX.add)
                    t = fresh(f"hlst_{c}_{m}", (P, 1))
                    nc.vector.tensor_copy(t, hs[:, NF - 1:NF])
                    hl_last[m] = t
                    if c < NCH - 1:
                        t = fresh(f"cplst_{c}_{m}", (P, 1))
                        nc.vector.tensor_copy(t, cp[:, NF - 1:NF])
                        cp_last[m] = t

                    yb = fresh(f"gf_{m}")
                    nc.vector.tensor_mul(yb, hs, cm_b)
                    yb2 = fresh(f"gf_{m}")
                    nc.vector.scalar_tensor_tensor(
                        yb2, xi[m], DPr[m], yb, AX.mult, AX.add)
                    t = fresh(f"yb3_{c}_{m}", bufs=1)
                    nc.vector.tensor_mul(t, yb2, sz[m])
                    yb3[c][m] = t
                    cf = fresh(f"gf_{m}")
                    nc.vector.tensor_mul(cf, cp, cm_b)
                    t = fresh(f"cf2_{c}_{m}", dt=bf16, bufs=1)
                    nc.vector.tensor_mul(t, cf, sz[m])
                    cf2[c][m] = t

            # ---- pairwise state exchange ----
            snd = fresh("snd", (P, KE), bufs=2)
            for m in range(KE):
                nc.vector.tensor_copy(snd[:, m:m + 1], hl_last[m])
            srecv = fresh("srecv", (P, KE), bufs=2)
            if no_cc:
                nc.vector.memset(srecv, 0.0)
            else:
                dma(g["cc_in"][ly], snd)
                nc.gpsimd.collective_compute(
                    "AllGather", AX.bypass, replica_groups=RG_PAIRS,
                    ins=[g["cc_in"][ly]], outs=[g["cc_out"][ly]])
                dma(srecv, g["cc_out"][ly][0])
            smask = fresh("smask", (P, KE), bufs=2)
            nc.vector.tensor_scalar_mul(smask, srecv, par_sb)

            # ---- finish y, out_proj, spill x ----
            for c in range(NCH):
                yh = [None] * KE
                yl = [None] * KE
                for m in range(KE):
                    yg = fresh(f"gf_{m}")
                    nc.vector.scalar_tensor_tensor(
                        yg, cf2[c][m], smask[:, m:m + 1], yb3[c][m],
                        AX.mult, AX.add)
                    t = fresh(f"yh_{m}", dt=bf16, bufs=1)
                    nc.vector.tensor_copy(t, yg)
                    yh[m] = t
                    t = fresh(f"yl_{m}", dt=bf16, bufs=1)
                    nc.vector.tensor_sub(t, yg, yh[m])
                    yl[m] = t
                for m in range(KD):
                    blks = load_blocks(g["w_out_hi"], g["w_out_lo"], m, KE)
                    ps = fresh("mm_ps", pool=psp, bufs=4)
                    mm3(ps, blks, yh, yl)
                    xt = fresh("xio", bufs=3)
                    nc.vector.tensor_copy(xt, ps)
                    dma(dst[m, :, c * NF:(c + 1) * NF], xt)

        # ---------------- output head ----------------
        NVCH = (V + NF - 1) // NF
        src = g["x_dram"][n_layers % 2]
        on = [None] * NCH
        for c in range(NCH):
            xs = load_x(src, c)
            on[c], _ = rmsnorm_cast(xs, ONW, False, f"on{c}")
        for vch in range(NVCH):
            nv = min(NF, V - vch * NF)
            wk = []
            for k in range(KD):
                t = fresh(f"whd_{k}", (P, NF), bf16, bufs=1)
                dma(t[:, 0:nv], g["w_head"][k * P:(k + 1) * P,
                                            vch * NF:vch * NF + nv])
                wk.append(t)
            hbr = fresh("hbr", (1, NF))
            dma(hbr[:, 0:nv], g["head_b"][:, vch * NF:vch * NF + nv])
            hbb = fresh("hbb", (P, NF))
            nc.gpsimd.partition_broadcast(hbb, hbr)
            for c in range(NCH):
                for mt in range(NF // P):
                    ps = fresh("mm_ps", pool=psp, bufs=4)
                    for k in range(KD):
                        nc.tensor.matmul(
                            ps[:, 0:nv],
                            on[c][k][:, mt * P:(mt + 1) * P],
                            wk[k][:, 0:nv],
                            start=(k == 0), stop=(k == KD - 1))
                    ot = fresh("ot", bufs=2)
                    nc.vector.scalar_tensor_tensor(
                        ot[:, 0:nv], ps[:, 0:nv], 1.0, hbb[:, 0:nv],
                        AX.mult, AX.add)
                    tok0 = c * NF + mt * P
                    dma(g["out"][tok0:tok0 + P, vch * NF:vch * NF + nv],
                        ot[:, 0:nv])


_CACHE = {}


def _get_program(n_layers=NLAYERS):
    if n_layers not in _CACHE:
        _CACHE[n_layers] = build_program(n_layers)
    return _CACHE[n_layers]


def _split(a):
    hi = a.astype(np_bf16)
    lo = (a - hi.astype(np.float32)).astype(np_bf16)
    return hi, lo


def _pad_wx(wx):
    out = np.zeros((ED, XCOLS), np.float32)
    out[:, BCOL] = wx[:, DT_RANK]
    out[:, CCOL] = wx[:, DT_RANK + 1]
    out[:, DTCOL:DTCOL + DT_RANK] = wx[:, :DT_RANK]
    return np.ascontiguousarray(out.astype(np_bf16))


def _blk(a, KR, KC):
    """[KR*128, KC*128] -> [KR, KC, 128, 128] contiguous blocks"""
    return np.ascontiguousarray(
        a.reshape(KR, P, KC, P).transpose(0, 2, 1, 3))


def prep_inputs(tokens, n_layers, emb_table, emb_proj_w, norm_w, in_proj_w,
                conv_w, conv_b, x_proj_w, dt_proj_w, dt_proj_b, A_log,
                D_param, out_proj_w, out_norm_w, head_w, head_b):
    tokens = np.asarray(tokens)
    emb_table = np.asarray(emb_table, np.float32)
    A0 = -np.exp(np.asarray(A_log, np.float32)[:, 0])
    pp = np.stack([
        np.asarray(norm_w, np.float32),
        np.asarray(conv_w, np.float32)[:, 0],
        np.asarray(conv_b, np.float32),
        A0,
        np.asarray(D_param, np.float32),
        np.asarray(dt_proj_b, np.float32),
        np.asarray(out_norm_w, np.float32),
        np.zeros(D, np.float32),
    ], axis=1)
    weh, wel = _split(np.asarray(emb_proj_w, np.float32))
    wih, wil = _split(np.asarray(in_proj_w, np.float32))
    woh, wol = _split(np.asarray(out_proj_w, np.float32))
    shared = dict(
        w_emb_hi=_blk(weh, KM, KD), w_emb_lo=_blk(wel, KM, KD),
        w_in_hi=_blk(wih, KD, 2 * KE), w_in_lo=_blk(wil, KD, 2 * KE),
        w_out_hi=_blk(woh, KE, KD), w_out_lo=_blk(wol, KE, KD),
        w_x=_pad_wx(np.asarray(x_proj_w, np.float32)),
        w_dt=np.ascontiguousarray(
            np.asarray(dt_proj_w, np.float32).astype(np_bf16)),
        w_head=np.ascontiguousarray(
            np.asarray(head_w, np.float32).astype(np_bf16)),
        pp=np.ascontiguousarray(pp),
        head_b=np.ascontiguousarray(np.asarray(head_b, np.float32)[None, :]),
    )
    in_maps = []
    for c in range(NCORES):
        b, half = c // 2, c % 2
        tok = tokens[b, half * T:(half + 1) * T]
        embT = np.ascontiguousarray(emb_table[tok].T)  # [512, 1024]
        ehi, elo = _split(embT)
        m = dict(shared)
        m["e_hi"] = np.ascontiguousarray(ehi)
        m["e_lo"] = np.ascontiguousarray(elo)
        m["parity"] = np.full((P, 1), float(half), np.float32)
        in_maps.append(m)
    return in_maps


LAST_RESULT = None


def kernel(**inputs):
    global LAST_RESULT
    n_layers = int(np.asarray(inputs["n_layers"]))
    assert n_layers == NLAYERS, f"hardcoded for {NLAYERS} layers, got {n_layers}"
    nc = _get_program(NLAYERS)
    in_maps = prep_inputs(**inputs)
    res = run_bass_kernel_spmd(nc, in_maps, core_ids=list(range(NCORES)))
    LAST_RESULT = res
    out = np.empty((B, L, V), np.float32)
    for c in range(NCORES):
        b, half = c // 2, c % 2
        out[b, half * T:(half + 1) * T, :] = res.results[c]["out"]
    return out


def build_stub():
    """Same I/O signature as build_program, trivial body — used to calibrate
    per-call dispatch overhead when timing via PJRT."""
    nc = bacc.Bacc("TRN2", target_bir_lowering=False, debug=False,
                   enable_asserts=False, num_devices=NCORES)

    def inp(name, shape, dt):
        return nc.dram_tensor(name, shape, dt, kind="ExternalInput").ap()

    e_hi = inp("e_hi", [EMB, T], bf16)
    e_lo = inp("e_lo", [EMB, T], bf16)
    inp("w_emb_hi", [KM, KD, P, P], bf16)
    inp("w_emb_lo", [KM, KD, P, P], bf16)
    inp("w_in_hi", [KD, 2 * KE, P, P], bf16)
    inp("w_in_lo", [KD, 2 * KE, P, P], bf16)
    inp("w_out_hi", [KE, KD, P, P], bf16)
    inp("w_out_lo", [KE, KD, P, P], bf16)
    inp("w_x", [ED, XCOLS], bf16)
    inp("w_dt", [DT_RANK, ED], bf16)
    inp("w_head", [D, V], bf16)
    inp("pp", [D, 8], f32)
    inp("head_b", [1, V], f32)
    parity = inp("parity", [P, 1], f32)
    out = nc.dram_tensor("out", [T, V], f32, kind="ExternalOutput").ap()

    with tile.TileContext(nc) as tc:
        with tc.tile_pool(name="p", bufs=1) as pool:
            t = pool.tile([P, 1], f32, name="t", tag="t")
            nc.sync.dma_start(t, parity)
            nc.sync.dma_start(out[0:P, 0:1], t)
    nc.compile()
    return nc

